# revision 1
# baseline (speedup 1.0000x reference)
"""GCN layer (GPSLayer) on 8 TRN2 NeuronCores — batched dma_gather design.

Math (matches reference):
  out[d] = dinv[d] * (x~[d] + sum_{e: dst=d} x~[src_e]) @ W_gcn
           + pos[d] @ W_pos + b_gcn + b_pos,   x~ = dinv * x

Design (the kernel is Q7/SWDGE descriptor-generation bound, ~7.6ns per
static index slot, so everything minimizes descriptor count):
 - x~ replicated in DRAM as 256B-padded f16 rows; per-edge source rows
   fetched by 36 large InstDMAGatherAnt calls (12-dst-tile group x 4
   int16-index segments of 25k nodes), every other engine hides under
   this chain.
 - greedy min-max node->tile assignment balances per-(tile,segment)
   counts across cores, so the SPMD-shared static call sizes sit ~0.7%
   above the true per-core edge count; host unpermutes output rows.
 - gathered chunks are scattered into per-dst-tile PSUM accumulators by
   one-hot matmuls (DVE-built from rel codes; rel=-1 kills padding),
   self loops are added directly from a sequential load, and the
   GCN/positional GEMMs + bias are fused into the per-tile tail.
"""

import numpy as np

from concourse import bacc, bass, mybir
import concourse.tile as tile
from concourse.bass_utils import run_bass_kernel_spmd
from concourse.library_config import mlp
from concourse.masks import make_identity

N_NODES = 100000
N_CORES = 8
D = 64
P = 128
ROW = 128          # padded row width (f16 elems) -> 256 B/row
NSEG = 4
GROUP = 12         # dst tiles per gather call

F16 = mybir.dt.float16
F32 = mybir.dt.float32
I16 = mybir.dt.int16

eq = mybir.AluOpType.is_equal
add_op = mybir.AluOpType.add


def _cdiv(a, b):
    return (a + b - 1) // b


def _plan(src, dst, n_nodes, n_cores):
    npc = n_nodes // n_cores
    n_tiles = _cdiv(npc, P)
    seg_rows = _cdiv(n_nodes, NSEG)
    assert seg_rows <= 32767
    xs_rows = seg_rows * NSEG
    n_groups = _cdiv(n_tiles, GROUP)

    deg = np.bincount(dst, minlength=n_nodes).astype(np.float64) + 1.0
    dinv = (1.0 / np.sqrt(deg)).astype(np.float32)

    # greedy min-max node -> (tile, row) assignment within each core:
    # each node carries its per-segment in-degree vector; place nodes
    # (largest first) into the tile minimizing the max per-segment load.
    # Gets per-(tile,seg) counts within ~1% of the perfect balance, which
    # minimizes the max-over-core static gather sizes.
    e_s_all = src // seg_rows
    dvec = np.zeros((n_nodes, NSEG), np.int64)
    np.add.at(dvec, (dst, e_s_all), 1)
    node_tile = np.empty(n_nodes, np.int64)
    node_row = np.empty(n_nodes, np.int64)
    cap_last = npc - (n_tiles - 1) * P
    for c in range(n_cores):
        nodes = np.arange(c * npc, (c + 1) * npc)
        order = nodes[np.argsort(-dvec[nodes].sum(axis=1), kind="stable")]
        cap = np.full(n_tiles, P, np.int64)
        cap[-1] = cap_last
        fill = np.zeros(n_tiles, np.int64)
        segsum = np.zeros((n_tiles, NSEG), np.float64)
        for n in order:
            cand = segsum + dvec[n]
            score = cand.max(axis=1) + 0.25 * cand.sum(axis=1)
            score[fill >= cap] = 1e18
            t = int(np.argmin(score))
            node_tile[n] = t
            node_row[n] = fill[t]
            segsum[t] += dvec[n]
            fill[t] += 1

    ne = len(src)
    e_s = e_s_all.astype(np.int64)
    e_srclocal = (src % seg_rows).astype(np.int16)
    e_core = dst // npc
    e_t = node_tile[dst]
    e_rel = node_row[dst].astype(np.float16)

    bucket = ((e_core * n_tiles + e_t) * NSEG + e_s)
    order = np.argsort(bucket, kind="stable")
    bucket = bucket[order]
    e_s, e_srclocal, e_core, e_t, e_rel = (
        e_s[order], e_srclocal[order], e_core[order], e_t[order], e_rel[order])

    nb = n_cores * n_tiles * NSEG
    counts = np.bincount(bucket, minlength=nb)
    starts = np.zeros(nb + 1, np.int64)
    np.cumsum(counts, out=starts[1:])
    pos = np.arange(ne) - starts[bucket]

    counts_cts = counts.reshape(n_cores, n_tiles, NSEG)
    # static idx slots per (t,s): max over cores, >=128 (keeps every SDMA
    # engine fed). No alignment needed: only the merged call's total
    # determines idx columns.
    stat = np.maximum(counts_cts.max(axis=0), P)

    g_of_t = np.arange(n_tiles) // GROUP
    # slot base of tile t inside its (g, s) call
    B = np.zeros((n_tiles, NSEG), np.int64)
    NI = np.zeros((n_groups, NSEG), np.int64)    # call sizes
    for g in range(n_groups):
        ts = np.nonzero(g_of_t == g)[0]
        B[ts] = np.concatenate(
            [np.zeros((1, NSEG), np.int64), np.cumsum(stat[ts], axis=0)[:-1]])
        NI[g] = stat[ts].sum(axis=0)
    nch_call = _cdiv(NI, P)                      # [G, S] chunks per call

    # idx col base per (g,s) call
    callcols = _cdiv(NI, 16)
    colbase = np.zeros((n_groups, NSEG), np.int64)
    flat = callcols.reshape(-1)
    cb = np.zeros(len(flat), np.int64)
    np.cumsum(flat[:-1], out=cb[1:])
    colbase[:, :] = cb.reshape(n_groups, NSEG)
    total_cols = int(flat.sum())

    # A-matrix columns: per (t, s): chunks overlapped by [B, B+stat)
    k0 = B // P                                  # first chunk
    k1 = (B + stat - 1) // P                     # last chunk
    novl = (k1 - k0 + 1).astype(np.int64)        # [T, S]
    tile_ncol = novl.sum(axis=1)                 # [T]
    tile_coloff = np.zeros(n_tiles + 1, np.int64)
    np.cumsum(tile_ncol, out=tile_coloff[1:])
    c_tot = int(tile_coloff[-1])
    s_coloff = np.zeros((n_tiles, NSEG), np.int64)
    s_coloff[:, 1:] = np.cumsum(novl, axis=1)[:, :-1]

    # per-edge placement
    q = B[e_t, e_s] + pos                        # slot within call
    e_icol = colbase[g_of_t[e_t], e_s] + q // 16
    e_ipart = (q % 16).astype(np.int64)
    e_chunk = q // P
    e_r = q % P
    # A column index: tile_coloff[t] + s_coloff[t,s] + (chunk - k0[t,s])
    e_acol = (tile_coloff[e_t] + s_coloff[e_t, e_s]
              + e_chunk - k0[e_t, e_s])

    meta = dict(npc=npc, n_tiles=n_tiles, seg_rows=seg_rows, xs_rows=xs_rows,
                n_groups=n_groups, stat=stat, B=B, NI=NI, nch_call=nch_call,
                colbase=colbase, total_cols=total_cols, k0=k0, novl=novl,
                tile_ncol=tile_ncol, tile_coloff=tile_coloff,
                s_coloff=s_coloff, c_tot=c_tot, dinv=dinv,
                node_tile=node_tile, node_row=node_row)

    per_core = []
    for c in range(n_cores):
        m = e_core == c
        idx16 = np.zeros((16, max(total_cols, 1)), np.int16)
        idx16[e_ipart[m], e_icol[m]] = e_srclocal[m]
        rel_all = np.full((P, max(c_tot, 1)), -1.0, np.float16)
        rel_all[e_r[m], e_acol[m]] = e_rel[m]
        per_core.append((np.ascontiguousarray(np.tile(idx16, (8, 1))),
                         np.ascontiguousarray(rel_all)))
    return meta, per_core


def _preprocess(x, edge_index, pos_encoding, W_gcn, b_gcn, W_pos, b_pos,
                n_nodes, n_cores):
    src = np.asarray(edge_index[0], dtype=np.int64)
    dst = np.asarray(edge_index[1], dtype=np.int64)
    meta, per_core_idx = _plan(src, dst, n_nodes, n_cores)
    npc, n_tiles = meta["npc"], meta["n_tiles"]
    nodes_pad = n_tiles * P
    dinv = meta["dinv"]

    x_s = np.zeros((meta["xs_rows"], ROW), np.float16)
    x_s[:n_nodes, :D] = (np.asarray(x, np.float32)
                         * dinv[:, None]).astype(np.float16)

    pos_f = np.asarray(pos_encoding, np.float32)
    node_tile, node_row = meta["node_tile"], meta["node_row"]
    # kernel output row (core c, tile t, row r) -> node id
    row_of_node = node_tile * P + node_row  # padded row within core
    per_core = []
    for c in range(n_cores):
        idx_arr, rel_all = per_core_idx[c]
        nodes = np.arange(c * npc, (c + 1) * npc)
        rpad = row_of_node[nodes]
        dv = np.zeros(nodes_pad, np.float32)
        dv[rpad] = dinv[nodes]
        dinv_tiles = np.ascontiguousarray(dv.reshape(n_tiles, P).T)
        pa = np.zeros((65, nodes_pad), np.float16)
        pa[:D, rpad] = pos_f[nodes].T.astype(np.float16)
        pa[D, rpad] = 1.0
        xo_full = np.zeros((nodes_pad, D), np.float16)
        xo_full[rpad] = x_s[nodes][:, :D]
        xo = np.ascontiguousarray(
            xo_full.reshape(n_tiles, P, D).transpose(1, 0, 2).reshape(
                P, n_tiles * D))
        per_core.append(dict(idx_all=idx_arr, rel_all=rel_all,
                             dinv_tiles=dinv_tiles,
                             posT=np.ascontiguousarray(pa),
                             xown=xo))

    b_sum = np.asarray(b_gcn, np.float32) + np.asarray(b_pos, np.float32)
    W_aug = np.zeros((65, D), np.float16)
    W_aug[:D] = np.asarray(W_pos, np.float32).astype(np.float16)
    W_aug[D] = b_sum.astype(np.float16)
    Wg16 = np.asarray(W_gcn, np.float32).astype(np.float16)

    shared = dict(x_s=x_s, W_gcn=Wg16, W_aug=W_aug)
    meta["row_of_node"] = row_of_node
    return shared, per_core, meta


def _build_program(meta):
    npc, n_tiles = meta["npc"], meta["n_tiles"]
    nodes_pad = n_tiles * P
    c_tot = meta["c_tot"]
    stat, B, NI, nch_call = (meta["stat"], meta["B"], meta["NI"],
                             meta["nch_call"])
    colbase, total_cols = meta["colbase"], meta["total_cols"]
    k0, novl = meta["k0"], meta["novl"]
    tile_ncol, tile_coloff = meta["tile_ncol"], meta["tile_coloff"]
    seg_rows, n_groups = meta["seg_rows"], meta["n_groups"]
    max_ncol = int(tile_ncol.max())
    nch_max = int(nch_call.max())

    nc = bacc.Bacc("TRN2", target_bir_lowering=False, debug=False)
    xs_d = nc.declare_dram_parameter(
        "x_s", [meta["xs_rows"], ROW], F16, isOutput=False)
    idx_d = nc.declare_dram_parameter(
        "idx_all", [P, max(total_cols, 1)], I16, isOutput=False)
    rel_d = nc.declare_dram_parameter(
        "rel_all", [P, max(c_tot, 1)], F16, isOutput=False)
    dinv_d = nc.declare_dram_parameter(
        "dinv_tiles", [P, n_tiles], F32, isOutput=False)
    posT_d = nc.declare_dram_parameter(
        "posT", [65, nodes_pad], F16, isOutput=False)
    xo_d = nc.declare_dram_parameter(
        "xown", [P, n_tiles * D], F16, isOutput=False)
    wg_d = nc.declare_dram_parameter("W_gcn", [D, D], F16, isOutput=False)
    wa_d = nc.declare_dram_parameter("W_aug", [65, D], F16, isOutput=False)
    out_d = nc.declare_dram_parameter("out", [nodes_pad, D], F32,
                                      isOutput=True)

    with tile.TileContext(nc) as tc:
        with (
            tc.tile_pool(name="const", bufs=1) as cpool,
            tc.tile_pool(name="msg", bufs=2) as mpool,
            tc.tile_pool(name="amat", bufs=3) as apool,
            tc.tile_pool(name="small", bufs=3) as spool,
            tc.tile_pool(name="outb", bufs=3) as opool,
            tc.tile_pool(name="ps_s", bufs=2, space="PSUM") as ps_s,
            tc.tile_pool(name="ps_t", bufs=2, space="PSUM") as ps_t,
            tc.tile_pool(name="ps_o", bufs=2, space="PSUM") as ps_o,
        ):
            iota_i = cpool.tile([P, P], mybir.dt.int16)
            nc.gpsimd.iota(iota_i[:], pattern=[[1, P]], base=0,
                           channel_multiplier=0)
            nc.gpsimd.load_library(mlp)
            iota_t = cpool.tile([P, P], F16)
            nc.vector.tensor_copy(out=iota_t[:], in_=iota_i[:])
            ident_t = cpool.tile([P, P], F16)
            make_identity(nc, ident_t[:])
            idx_ts = []
            for g in range(n_groups):
                g_lo = int(colbase[g, 0])
                g_cols = int(_cdiv(NI[g], 16).sum())
                it = cpool.tile([P, max(g_cols, 1)], I16, name=f"idxg{g}")
                nc.sync.dma_start(out=it[:],
                                  in_=idx_d[:, g_lo:g_lo + g_cols])
                idx_ts.append((it, g_lo))
            wg_t = cpool.tile([D, D], F16)
            nc.sync.dma_start(out=wg_t[:], in_=wg_d[:])
            wa_t = cpool.tile([65, D], F16)
            nc.sync.dma_start(out=wa_t[:], in_=wa_d[:])
            dinv_t = cpool.tile([P, n_tiles], F32)
            nc.sync.dma_start(out=dinv_t[:], in_=dinv_d[:])
            posT_t = cpool.tile([65, nodes_pad], F16)
            nc.sync.dma_start(out=posT_t[:], in_=posT_d[:])
            xo_t = cpool.tile([P, n_tiles * D], F16)
            nc.sync.dma_start(out=xo_t[:], in_=xo_d[:])
            rel_all = cpool.tile([P, max(c_tot, 1)], F16)
            nc.sync.dma_start(out=rel_all[:], in_=rel_d[:])

            for g in range(n_groups):
                t_lo = g * GROUP
                t_hi = min((g + 1) * GROUP, n_tiles)
                msg_of_s = {}
                for s in range(NSEG):
                    ni = int(NI[g, s])
                    nch = int(nch_call[g, s])
                    msg = mpool.tile([P, nch_max, ROW], F16, tag=f"m{s}",
                                     name=f"msg{s}")
                    if ni < nch * P:
                        nc.scalar.memzero(msg[:, nch - 1, :])
                    it, g_lo = idx_ts[g]
                    cb = int(colbase[g, s]) - g_lo
                    nc.gpsimd.dma_gather(
                        msg[:, :nch, :],
                        xs_d[s * seg_rows:(s + 1) * seg_rows, :],
                        it[:, cb:cb + _cdiv(ni, 16)],
                        ni,
                        ni,
                        ROW,
                        elem_step=ROW,
                        single_packet=False,
                    )
                    msg_of_s[s] = msg

                for t in range(t_lo, t_hi):
                    ncol = int(tile_ncol[t])
                    j0 = int(tile_coloff[t])
                    a_big = apool.tile([P, max_ncol, P], F16, tag="a_big")
                    nc.vector.tensor_tensor(
                        out=a_big[:, :ncol, :],
                        in0=rel_all[:, j0:j0 + ncol].unsqueeze(2)
                            .to_broadcast([P, ncol, P]),
                        in1=iota_t[:].unsqueeze(1).to_broadcast([P, ncol, P]),
                        op=eq)
                    psum_s = ps_s.tile([P, D], F32)
                    j = 0
                    for s in range(NSEG):
                        kk0 = int(k0[t, s])
                        for k in range(int(novl[t, s])):
                            nc.tensor.matmul(
                                out=psum_s[:],
                                lhsT=a_big[:, j, :],
                                rhs=msg_of_s[s][:, kk0 + k, :D],
                                start=(j == 0), stop=(j == ncol - 1))
                            j += 1

                    ssum = spool.tile([P, D], F32, tag="ssum")
                    nc.vector.tensor_tensor(
                        out=ssum[:], in0=psum_s[:],
                        in1=xo_t[:, t * D:(t + 1) * D], op=add_op)
                    s16 = spool.tile([P, D], F16, tag="s16")
                    nc.scalar.mul(out=s16[:], in_=ssum[:],
                                  mul=dinv_t[:, t:t + 1])
                    psT = ps_t.tile([D, P], F16)
                    nc.tensor.transpose(out=psT[:], in_=s16[:],
                                        identity=ident_t[:])
                    sT = spool.tile([D, P], F16, tag="sT")
                    nc.scalar.copy(out=sT[:], in_=psT[:])
                    psum_o = ps_o.tile([P, D], F32)
                    nc.tensor.matmul(out=psum_o[:], lhsT=sT[:], rhs=wg_t[:],
                                     start=True, stop=False)
                    nc.tensor.matmul(out=psum_o[:],
                                     lhsT=posT_t[:, t * P:(t + 1) * P],
                                     rhs=wa_t[:], start=False, stop=True)
                    out_sb = opool.tile([P, D], F32)
                    nc.scalar.copy(out=out_sb[:], in_=psum_o[:])
                    nc.sync.dma_start(out=out_d[t * P:(t + 1) * P, :],
                                      in_=out_sb[:])
    nc.compile()
    return nc


def kernel(x, edge_index, pos_encoding, W_gcn, b_gcn, W_pos, b_pos,
           _trace=False, _result_box=None, _tmpdir=None,
           _n_nodes=N_NODES, _n_cores=N_CORES):
    shared, per_core, meta = _preprocess(
        x, edge_index, pos_encoding, W_gcn, b_gcn, W_pos, b_pos,
        _n_nodes, _n_cores)
    nc = _build_program(meta)
    in_maps = [{**shared, **per_core[c]} for c in range(_n_cores)]
    res = run_bass_kernel_spmd(nc, in_maps, list(range(_n_cores)),
                               trace=_trace, tmpdir=_tmpdir)
    if _result_box is not None:
        _result_box.append(res)
    npc = meta["npc"]
    row_of_node = meta["row_of_node"]
    out = np.empty((_n_nodes, D), np.float32)
    for c in range(_n_cores):
        nodes = np.arange(c * npc, (c + 1) * npc)
        out[nodes] = np.asarray(res.results[c]["out"],
                                np.float32)[row_of_node[nodes]]
    return out


if __name__ == "__main__":
    rng = np.random.default_rng(0)
    x = rng.standard_normal((N_NODES, D), dtype=np.float32)
    ei = rng.integers(0, N_NODES, size=(2, 1600000)).astype(np.int64)
    pe = rng.standard_normal((N_NODES, D), dtype=np.float32)
    Wg = rng.standard_normal((D, D), dtype=np.float32) / 8
    bg = rng.standard_normal(D, dtype=np.float32) * 0.01
    Wp = rng.standard_normal((D, D), dtype=np.float32) / 8
    bp = rng.standard_normal(D, dtype=np.float32) * 0.01
    out = kernel(x, ei, pe, Wg, bg, Wp, bp)
    print(out.shape, out.dtype)



# revision 2
# speedup vs baseline: 1.1170x; 1.1170x over previous
"""GCN layer (GPSLayer) on 8 TRN2 NeuronCores — batched dma_gather design.

Math (matches reference):
  out[d] = dinv[d] * (x~[d] + sum_{e: dst=d} x~[src_e]) @ W_gcn
           + pos[d] @ W_pos + b_gcn + b_pos,   x~ = dinv * x

Design (the kernel is SWDGE-drain bound: each swdge queue's descriptors
drain at ~8ns/desc and 4 queues drain ~3x faster in aggregate, so the
gathers are spread across all 4 queues (queue_num=segment) and
everything minimizes descriptor count):
 - x~ replicated in DRAM as 256B-padded f16 rows; per-edge source rows
   fetched by 36 large InstDMAGatherAnt calls (12-dst-tile group x 4
   int16-index segments of 25k nodes), every other engine hides under
   this chain.
 - greedy min-max node->tile assignment balances per-(tile,segment)
   counts across cores, so the SPMD-shared static call sizes sit ~0.7%
   above the true per-core edge count; host unpermutes output rows.
 - gathered chunks are scattered into per-dst-tile PSUM accumulators by
   one-hot matmuls (DVE-built from rel codes; rel=-1 kills padding),
   self loops are added directly from a sequential load, and the
   GCN/positional GEMMs + bias are fused into the per-tile tail.
"""

import numpy as np

from concourse import bacc, bass, mybir
import concourse.tile as tile
from concourse.bass_utils import run_bass_kernel_spmd
from concourse.library_config import mlp
from concourse.masks import make_identity

N_NODES = 100000
N_CORES = 8
D = 64
P = 128
ROW = 128          # padded row width (f16 elems) -> 256 B/row
NSEG = 4
GROUP = 12         # dst tiles per gather call

F16 = mybir.dt.float16
F32 = mybir.dt.float32
I16 = mybir.dt.int16

eq = mybir.AluOpType.is_equal
add_op = mybir.AluOpType.add


def _cdiv(a, b):
    return (a + b - 1) // b


def _plan(src, dst, n_nodes, n_cores):
    npc = n_nodes // n_cores
    n_tiles = _cdiv(npc, P)
    seg_rows = _cdiv(n_nodes, NSEG)
    assert seg_rows <= 32767
    xs_rows = seg_rows * NSEG
    n_groups = _cdiv(n_tiles, GROUP)

    deg = np.bincount(dst, minlength=n_nodes).astype(np.float64) + 1.0
    dinv = (1.0 / np.sqrt(deg)).astype(np.float32)

    # greedy min-max node -> (tile, row) assignment within each core:
    # each node carries its per-segment in-degree vector; place nodes
    # (largest first) into the tile minimizing the max per-segment load.
    # Gets per-(tile,seg) counts within ~1% of the perfect balance, which
    # minimizes the max-over-core static gather sizes.
    e_s_all = src // seg_rows
    dvec = np.zeros((n_nodes, NSEG), np.int64)
    np.add.at(dvec, (dst, e_s_all), 1)
    node_tile = np.empty(n_nodes, np.int64)
    node_row = np.empty(n_nodes, np.int64)
    cap_last = npc - (n_tiles - 1) * P
    for c in range(n_cores):
        nodes = np.arange(c * npc, (c + 1) * npc)
        order = nodes[np.argsort(-dvec[nodes].sum(axis=1), kind="stable")]
        cap = np.full(n_tiles, P, np.int64)
        cap[-1] = cap_last
        fill = np.zeros(n_tiles, np.int64)
        segsum = np.zeros((n_tiles, NSEG), np.float64)
        for n in order:
            cand = segsum + dvec[n]
            score = cand.max(axis=1) + 0.25 * cand.sum(axis=1)
            score[fill >= cap] = 1e18
            t = int(np.argmin(score))
            node_tile[n] = t
            node_row[n] = fill[t]
            segsum[t] += dvec[n]
            fill[t] += 1

    ne = len(src)
    e_s = e_s_all.astype(np.int64)
    e_srclocal = (src % seg_rows).astype(np.int16)
    e_core = dst // npc
    e_t = node_tile[dst]
    e_rel = node_row[dst].astype(np.float16)

    bucket = ((e_core * n_tiles + e_t) * NSEG + e_s)
    order = np.argsort(bucket, kind="stable")
    bucket = bucket[order]
    e_s, e_srclocal, e_core, e_t, e_rel = (
        e_s[order], e_srclocal[order], e_core[order], e_t[order], e_rel[order])

    nb = n_cores * n_tiles * NSEG
    counts = np.bincount(bucket, minlength=nb)
    starts = np.zeros(nb + 1, np.int64)
    np.cumsum(counts, out=starts[1:])
    pos = np.arange(ne) - starts[bucket]

    counts_cts = counts.reshape(n_cores, n_tiles, NSEG)
    # static idx slots per (t,s): max over cores, >=128 (keeps every SDMA
    # engine fed). No alignment needed: only the merged call's total
    # determines idx columns.
    stat = np.maximum(counts_cts.max(axis=0), P)

    g_of_t = np.arange(n_tiles) // GROUP
    # slot base of tile t inside its (g, s) call
    B = np.zeros((n_tiles, NSEG), np.int64)
    NI = np.zeros((n_groups, NSEG), np.int64)    # call sizes
    for g in range(n_groups):
        ts = np.nonzero(g_of_t == g)[0]
        B[ts] = np.concatenate(
            [np.zeros((1, NSEG), np.int64), np.cumsum(stat[ts], axis=0)[:-1]])
        NI[g] = stat[ts].sum(axis=0)
    nch_call = _cdiv(NI, P)                      # [G, S] chunks per call

    # idx col base per (g,s) call
    callcols = _cdiv(NI, 16)
    colbase = np.zeros((n_groups, NSEG), np.int64)
    flat = callcols.reshape(-1)
    cb = np.zeros(len(flat), np.int64)
    np.cumsum(flat[:-1], out=cb[1:])
    colbase[:, :] = cb.reshape(n_groups, NSEG)
    total_cols = int(flat.sum())

    # A-matrix columns: per (t, s): chunks overlapped by [B, B+stat)
    k0 = B // P                                  # first chunk
    k1 = (B + stat - 1) // P                     # last chunk
    novl = (k1 - k0 + 1).astype(np.int64)        # [T, S]
    tile_ncol = novl.sum(axis=1)                 # [T]
    tile_coloff = np.zeros(n_tiles + 1, np.int64)
    np.cumsum(tile_ncol, out=tile_coloff[1:])
    c_tot = int(tile_coloff[-1])
    s_coloff = np.zeros((n_tiles, NSEG), np.int64)
    s_coloff[:, 1:] = np.cumsum(novl, axis=1)[:, :-1]

    # per-edge placement
    q = B[e_t, e_s] + pos                        # slot within call
    e_icol = colbase[g_of_t[e_t], e_s] + q // 16
    e_ipart = (q % 16).astype(np.int64)
    e_chunk = q // P
    e_r = q % P
    # A column index: tile_coloff[t] + s_coloff[t,s] + (chunk - k0[t,s])
    e_acol = (tile_coloff[e_t] + s_coloff[e_t, e_s]
              + e_chunk - k0[e_t, e_s])

    meta = dict(npc=npc, n_tiles=n_tiles, seg_rows=seg_rows, xs_rows=xs_rows,
                n_groups=n_groups, stat=stat, B=B, NI=NI, nch_call=nch_call,
                colbase=colbase, total_cols=total_cols, k0=k0, novl=novl,
                tile_ncol=tile_ncol, tile_coloff=tile_coloff,
                s_coloff=s_coloff, c_tot=c_tot, dinv=dinv,
                node_tile=node_tile, node_row=node_row)

    per_core = []
    for c in range(n_cores):
        m = e_core == c
        idx16 = np.zeros((16, max(total_cols, 1)), np.int16)
        idx16[e_ipart[m], e_icol[m]] = e_srclocal[m]
        rel_all = np.full((P, max(c_tot, 1)), -1.0, np.float16)
        rel_all[e_r[m], e_acol[m]] = e_rel[m]
        per_core.append((np.ascontiguousarray(np.tile(idx16, (8, 1))),
                         np.ascontiguousarray(rel_all)))
    return meta, per_core


def _preprocess(x, edge_index, pos_encoding, W_gcn, b_gcn, W_pos, b_pos,
                n_nodes, n_cores):
    src = np.asarray(edge_index[0], dtype=np.int64)
    dst = np.asarray(edge_index[1], dtype=np.int64)
    meta, per_core_idx = _plan(src, dst, n_nodes, n_cores)
    npc, n_tiles = meta["npc"], meta["n_tiles"]
    nodes_pad = n_tiles * P
    dinv = meta["dinv"]

    x_s = np.zeros((meta["xs_rows"], ROW), np.float16)
    x_s[:n_nodes, :D] = (np.asarray(x, np.float32)
                         * dinv[:, None]).astype(np.float16)

    pos_f = np.asarray(pos_encoding, np.float32)
    node_tile, node_row = meta["node_tile"], meta["node_row"]
    # kernel output row (core c, tile t, row r) -> node id
    row_of_node = node_tile * P + node_row  # padded row within core
    per_core = []
    for c in range(n_cores):
        idx_arr, rel_all = per_core_idx[c]
        nodes = np.arange(c * npc, (c + 1) * npc)
        rpad = row_of_node[nodes]
        dv = np.zeros(nodes_pad, np.float32)
        dv[rpad] = dinv[nodes]
        dinv_tiles = np.ascontiguousarray(dv.reshape(n_tiles, P).T)
        pa = np.zeros((65, nodes_pad), np.float16)
        pa[:D, rpad] = pos_f[nodes].T.astype(np.float16)
        pa[D, rpad] = 1.0
        xo_full = np.zeros((nodes_pad, D), np.float16)
        xo_full[rpad] = x_s[nodes][:, :D]
        xo = np.ascontiguousarray(
            xo_full.reshape(n_tiles, P, D).transpose(1, 0, 2).reshape(
                P, n_tiles * D))
        per_core.append(dict(idx_all=idx_arr, rel_all=rel_all,
                             dinv_tiles=dinv_tiles,
                             posT=np.ascontiguousarray(pa),
                             xown=xo))

    b_sum = np.asarray(b_gcn, np.float32) + np.asarray(b_pos, np.float32)
    W_aug = np.zeros((65, D), np.float16)
    W_aug[:D] = np.asarray(W_pos, np.float32).astype(np.float16)
    W_aug[D] = b_sum.astype(np.float16)
    Wg16 = np.asarray(W_gcn, np.float32).astype(np.float16)

    shared = dict(x_s=x_s, W_gcn=Wg16, W_aug=W_aug)
    meta["row_of_node"] = row_of_node
    return shared, per_core, meta


def _build_program(meta):
    npc, n_tiles = meta["npc"], meta["n_tiles"]
    nodes_pad = n_tiles * P
    c_tot = meta["c_tot"]
    stat, B, NI, nch_call = (meta["stat"], meta["B"], meta["NI"],
                             meta["nch_call"])
    colbase, total_cols = meta["colbase"], meta["total_cols"]
    k0, novl = meta["k0"], meta["novl"]
    tile_ncol, tile_coloff = meta["tile_ncol"], meta["tile_coloff"]
    seg_rows, n_groups = meta["seg_rows"], meta["n_groups"]
    max_ncol = int(tile_ncol.max())
    nch_max = int(nch_call.max())

    nc = bacc.Bacc("TRN2", target_bir_lowering=False, debug=False,
                   num_swdge_queues=4)
    xs_d = nc.declare_dram_parameter(
        "x_s", [meta["xs_rows"], ROW], F16, isOutput=False)
    idx_d = nc.declare_dram_parameter(
        "idx_all", [P, max(total_cols, 1)], I16, isOutput=False)
    rel_d = nc.declare_dram_parameter(
        "rel_all", [P, max(c_tot, 1)], F16, isOutput=False)
    dinv_d = nc.declare_dram_parameter(
        "dinv_tiles", [P, n_tiles], F32, isOutput=False)
    posT_d = nc.declare_dram_parameter(
        "posT", [65, nodes_pad], F16, isOutput=False)
    xo_d = nc.declare_dram_parameter(
        "xown", [P, n_tiles * D], F16, isOutput=False)
    wg_d = nc.declare_dram_parameter("W_gcn", [D, D], F16, isOutput=False)
    wa_d = nc.declare_dram_parameter("W_aug", [65, D], F16, isOutput=False)
    out_d = nc.declare_dram_parameter("out", [nodes_pad, D], F32,
                                      isOutput=True)

    with tile.TileContext(nc) as tc:
        with (
            tc.tile_pool(name="const", bufs=1) as cpool,
            tc.tile_pool(name="msg", bufs=2) as mpool,
            tc.tile_pool(name="amat", bufs=3) as apool,
            tc.tile_pool(name="small", bufs=3) as spool,
            tc.tile_pool(name="outb", bufs=3) as opool,
            tc.tile_pool(name="ps_s", bufs=2, space="PSUM") as ps_s,
            tc.tile_pool(name="ps_t", bufs=2, space="PSUM") as ps_t,
            tc.tile_pool(name="ps_o", bufs=2, space="PSUM") as ps_o,
        ):
            iota_i = cpool.tile([P, P], mybir.dt.int16)
            nc.gpsimd.iota(iota_i[:], pattern=[[1, P]], base=0,
                           channel_multiplier=0)
            nc.gpsimd.load_library(mlp)
            iota_t = cpool.tile([P, P], F16)
            nc.vector.tensor_copy(out=iota_t[:], in_=iota_i[:])
            ident_t = cpool.tile([P, P], F16)
            make_identity(nc, ident_t[:])
            idx_ts = []
            for g in range(n_groups):
                g_lo = int(colbase[g, 0])
                g_cols = int(_cdiv(NI[g], 16).sum())
                it = cpool.tile([P, max(g_cols, 1)], I16, name=f"idxg{g}")
                nc.sync.dma_start(out=it[:],
                                  in_=idx_d[:, g_lo:g_lo + g_cols])
                idx_ts.append((it, g_lo))
            wg_t = cpool.tile([D, D], F16)
            nc.sync.dma_start(out=wg_t[:], in_=wg_d[:])
            wa_t = cpool.tile([65, D], F16)
            nc.sync.dma_start(out=wa_t[:], in_=wa_d[:])
            dinv_t = cpool.tile([P, n_tiles], F32)
            nc.sync.dma_start(out=dinv_t[:], in_=dinv_d[:])
            posT_t = cpool.tile([65, nodes_pad], F16)
            nc.sync.dma_start(out=posT_t[:], in_=posT_d[:])
            xo_t = cpool.tile([P, n_tiles * D], F16)
            nc.sync.dma_start(out=xo_t[:], in_=xo_d[:])
            rel_all = cpool.tile([P, max(c_tot, 1)], F16)
            nc.sync.dma_start(out=rel_all[:], in_=rel_d[:])

            for g in range(n_groups):
                t_lo = g * GROUP
                t_hi = min((g + 1) * GROUP, n_tiles)
                msg_of_s = {}
                for s in range(NSEG):
                    ni = int(NI[g, s])
                    nch = int(nch_call[g, s])
                    msg = mpool.tile([P, nch_max, ROW], F16, tag=f"m{s}",
                                     name=f"msg{s}")
                    if ni < nch * P:
                        nc.scalar.memzero(msg[:, nch - 1, :])
                    it, g_lo = idx_ts[g]
                    cb = int(colbase[g, s]) - g_lo
                    nc.gpsimd.dma_gather(
                        msg[:, :nch, :],
                        xs_d[s * seg_rows:(s + 1) * seg_rows, :],
                        it[:, cb:cb + _cdiv(ni, 16)],
                        ni,
                        ni,
                        ROW,
                        elem_step=ROW,
                        single_packet=False,
                        queue_num=s,
                    )
                    msg_of_s[s] = msg

                for t in range(t_lo, t_hi):
                    ncol = int(tile_ncol[t])
                    j0 = int(tile_coloff[t])
                    a_big = apool.tile([P, max_ncol, P], F16, tag="a_big")
                    nc.vector.tensor_tensor(
                        out=a_big[:, :ncol, :],
                        in0=rel_all[:, j0:j0 + ncol].unsqueeze(2)
                            .to_broadcast([P, ncol, P]),
                        in1=iota_t[:].unsqueeze(1).to_broadcast([P, ncol, P]),
                        op=eq)
                    psum_s = ps_s.tile([P, D], F32)
                    j = 0
                    for s in range(NSEG):
                        kk0 = int(k0[t, s])
                        for k in range(int(novl[t, s])):
                            nc.tensor.matmul(
                                out=psum_s[:],
                                lhsT=a_big[:, j, :],
                                rhs=msg_of_s[s][:, kk0 + k, :D],
                                start=(j == 0), stop=(j == ncol - 1))
                            j += 1

                    ssum = spool.tile([P, D], F32, tag="ssum")
                    nc.vector.tensor_tensor(
                        out=ssum[:], in0=psum_s[:],
                        in1=xo_t[:, t * D:(t + 1) * D], op=add_op)
                    s16 = spool.tile([P, D], F16, tag="s16")
                    nc.scalar.mul(out=s16[:], in_=ssum[:],
                                  mul=dinv_t[:, t:t + 1])
                    psT = ps_t.tile([D, P], F16)
                    nc.tensor.transpose(out=psT[:], in_=s16[:],
                                        identity=ident_t[:])
                    sT = spool.tile([D, P], F16, tag="sT")
                    nc.scalar.copy(out=sT[:], in_=psT[:])
                    psum_o = ps_o.tile([P, D], F32)
                    nc.tensor.matmul(out=psum_o[:], lhsT=sT[:], rhs=wg_t[:],
                                     start=True, stop=False)
                    nc.tensor.matmul(out=psum_o[:],
                                     lhsT=posT_t[:, t * P:(t + 1) * P],
                                     rhs=wa_t[:], start=False, stop=True)
                    out_sb = opool.tile([P, D], F32)
                    nc.scalar.copy(out=out_sb[:], in_=psum_o[:])
                    nc.sync.dma_start(out=out_d[t * P:(t + 1) * P, :],
                                      in_=out_sb[:])
    nc.compile()
    return nc


def kernel(x, edge_index, pos_encoding, W_gcn, b_gcn, W_pos, b_pos,
           _trace=False, _result_box=None, _tmpdir=None,
           _n_nodes=N_NODES, _n_cores=N_CORES):
    shared, per_core, meta = _preprocess(
        x, edge_index, pos_encoding, W_gcn, b_gcn, W_pos, b_pos,
        _n_nodes, _n_cores)
    nc = _build_program(meta)
    in_maps = [{**shared, **per_core[c]} for c in range(_n_cores)]
    res = run_bass_kernel_spmd(nc, in_maps, list(range(_n_cores)),
                               trace=_trace, tmpdir=_tmpdir)
    if _result_box is not None:
        _result_box.append(res)
    npc = meta["npc"]
    row_of_node = meta["row_of_node"]
    out = np.empty((_n_nodes, D), np.float32)
    for c in range(_n_cores):
        nodes = np.arange(c * npc, (c + 1) * npc)
        out[nodes] = np.asarray(res.results[c]["out"],
                                np.float32)[row_of_node[nodes]]
    return out


if __name__ == "__main__":
    rng = np.random.default_rng(0)
    x = rng.standard_normal((N_NODES, D), dtype=np.float32)
    ei = rng.integers(0, N_NODES, size=(2, 1600000)).astype(np.int64)
    pe = rng.standard_normal((N_NODES, D), dtype=np.float32)
    Wg = rng.standard_normal((D, D), dtype=np.float32) / 8
    bg = rng.standard_normal(D, dtype=np.float32) * 0.01
    Wp = rng.standard_normal((D, D), dtype=np.float32) / 8
    bp = rng.standard_normal(D, dtype=np.float32) * 0.01
    out = kernel(x, ei, pe, Wg, bg, Wp, bp)
    print(out.shape, out.dtype)



# revision 3
# speedup vs baseline: 1.1348x; 1.0159x over previous
"""GCN layer (GPSLayer) on 8 TRN2 NeuronCores — batched dma_gather design.

Math (matches reference):
  out[d] = dinv[d] * (x~[d] + sum_{e: dst=d} x~[src_e]) @ W_gcn
           + pos[d] @ W_pos + b_gcn + b_pos,   x~ = dinv * x

Design (the kernel is SWDGE-drain bound: each swdge queue's descriptors
drain at ~8ns/desc and 4 queues drain ~3x faster in aggregate, so the
gathers are spread across all 4 queues (queue_num=segment) and
everything minimizes descriptor count):
 - x~ replicated in DRAM as 256B-padded f16 rows; per-edge source rows
   fetched by 36 large InstDMAGatherAnt calls (12-dst-tile group x 4
   int16-index segments of 25k nodes), every other engine hides under
   this chain.
 - greedy min-max node->tile assignment balances per-(tile,segment)
   counts across cores, so the SPMD-shared static call sizes sit ~0.7%
   above the true per-core edge count; host unpermutes output rows.
 - gathered chunks are scattered into per-dst-tile PSUM accumulators by
   one-hot matmuls (DVE-built from rel codes; rel=-1 kills padding),
   self loops are added directly from a sequential load, and the
   GCN/positional GEMMs + bias are fused into the per-tile tail.
"""

import numpy as np

from concourse import bacc, bass, mybir
import concourse.tile as tile
from concourse.bass_utils import run_bass_kernel_spmd
from concourse.library_config import mlp
from concourse.masks import make_identity

N_NODES = 100000
N_CORES = 8
D = 64
P = 128
ROW = 128          # padded row width (f16 elems) -> 256 B/row
NSEG = 4
GROUP = 6          # dst tiles per gather call

F16 = mybir.dt.float16
F32 = mybir.dt.float32
I16 = mybir.dt.int16

eq = mybir.AluOpType.is_equal
add_op = mybir.AluOpType.add


def _cdiv(a, b):
    return (a + b - 1) // b


def _plan(src, dst, n_nodes, n_cores):
    npc = n_nodes // n_cores
    n_tiles = _cdiv(npc, P)
    seg_rows = _cdiv(n_nodes, NSEG)
    assert seg_rows <= 32767
    xs_rows = seg_rows * NSEG
    n_groups = _cdiv(n_tiles, GROUP)

    deg = np.bincount(dst, minlength=n_nodes).astype(np.float64) + 1.0
    dinv = (1.0 / np.sqrt(deg)).astype(np.float32)

    # greedy min-max node -> (tile, row) assignment within each core:
    # each node carries its per-segment in-degree vector; place nodes
    # (largest first) into the tile minimizing the max per-segment load.
    # Gets per-(tile,seg) counts within ~1% of the perfect balance, which
    # minimizes the max-over-core static gather sizes.
    e_s_all = src // seg_rows
    dvec = np.zeros((n_nodes, NSEG), np.int64)
    np.add.at(dvec, (dst, e_s_all), 1)
    node_tile = np.empty(n_nodes, np.int64)
    node_row = np.empty(n_nodes, np.int64)
    cap_last = npc - (n_tiles - 1) * P
    for c in range(n_cores):
        nodes = np.arange(c * npc, (c + 1) * npc)
        order = nodes[np.argsort(-dvec[nodes].sum(axis=1), kind="stable")]
        cap = np.full(n_tiles, P, np.int64)
        cap[-1] = cap_last
        fill = np.zeros(n_tiles, np.int64)
        segsum = np.zeros((n_tiles, NSEG), np.float64)
        for n in order:
            cand = segsum + dvec[n]
            score = cand.max(axis=1) + 0.25 * cand.sum(axis=1)
            score[fill >= cap] = 1e18
            t = int(np.argmin(score))
            node_tile[n] = t
            node_row[n] = fill[t]
            segsum[t] += dvec[n]
            fill[t] += 1

    ne = len(src)
    e_s = e_s_all.astype(np.int64)
    e_srclocal = (src % seg_rows).astype(np.int16)
    e_core = dst // npc
    e_t = node_tile[dst]
    e_rel = node_row[dst].astype(np.float16)

    bucket = ((e_core * n_tiles + e_t) * NSEG + e_s)
    order = np.argsort(bucket, kind="stable")
    bucket = bucket[order]
    e_s, e_srclocal, e_core, e_t, e_rel = (
        e_s[order], e_srclocal[order], e_core[order], e_t[order], e_rel[order])

    nb = n_cores * n_tiles * NSEG
    counts = np.bincount(bucket, minlength=nb)
    starts = np.zeros(nb + 1, np.int64)
    np.cumsum(counts, out=starts[1:])
    pos = np.arange(ne) - starts[bucket]

    counts_cts = counts.reshape(n_cores, n_tiles, NSEG)
    # static idx slots per (t,s): max over cores, >=128 (keeps every SDMA
    # engine fed). No alignment needed: only the merged call's total
    # determines idx columns.
    stat = np.maximum(counts_cts.max(axis=0), P)

    g_of_t = np.arange(n_tiles) // GROUP
    # slot base of tile t inside its (g, s) call
    B = np.zeros((n_tiles, NSEG), np.int64)
    NI = np.zeros((n_groups, NSEG), np.int64)    # call sizes
    for g in range(n_groups):
        ts = np.nonzero(g_of_t == g)[0]
        B[ts] = np.concatenate(
            [np.zeros((1, NSEG), np.int64), np.cumsum(stat[ts], axis=0)[:-1]])
        NI[g] = stat[ts].sum(axis=0)
    nch_call = _cdiv(NI, P)                      # [G, S] chunks per call

    # idx col base per (g,s) call
    callcols = _cdiv(NI, 16)
    colbase = np.zeros((n_groups, NSEG), np.int64)
    flat = callcols.reshape(-1)
    cb = np.zeros(len(flat), np.int64)
    np.cumsum(flat[:-1], out=cb[1:])
    colbase[:, :] = cb.reshape(n_groups, NSEG)
    total_cols = int(flat.sum())

    # A-matrix columns: per (t, s): chunks overlapped by [B, B+stat)
    k0 = B // P                                  # first chunk
    k1 = (B + stat - 1) // P                     # last chunk
    novl = (k1 - k0 + 1).astype(np.int64)        # [T, S]
    tile_ncol = novl.sum(axis=1)                 # [T]
    tile_coloff = np.zeros(n_tiles + 1, np.int64)
    np.cumsum(tile_ncol, out=tile_coloff[1:])
    c_tot = int(tile_coloff[-1])
    s_coloff = np.zeros((n_tiles, NSEG), np.int64)
    s_coloff[:, 1:] = np.cumsum(novl, axis=1)[:, :-1]

    # per-edge placement
    q = B[e_t, e_s] + pos                        # slot within call
    e_icol = colbase[g_of_t[e_t], e_s] + q // 16
    e_ipart = (q % 16).astype(np.int64)
    e_chunk = q // P
    e_r = q % P
    # A column index: tile_coloff[t] + s_coloff[t,s] + (chunk - k0[t,s])
    e_acol = (tile_coloff[e_t] + s_coloff[e_t, e_s]
              + e_chunk - k0[e_t, e_s])

    meta = dict(npc=npc, n_tiles=n_tiles, seg_rows=seg_rows, xs_rows=xs_rows,
                n_groups=n_groups, stat=stat, B=B, NI=NI, nch_call=nch_call,
                colbase=colbase, total_cols=total_cols, k0=k0, novl=novl,
                tile_ncol=tile_ncol, tile_coloff=tile_coloff,
                s_coloff=s_coloff, c_tot=c_tot, dinv=dinv,
                node_tile=node_tile, node_row=node_row)

    per_core = []
    for c in range(n_cores):
        m = e_core == c
        idx16 = np.zeros((16, max(total_cols, 1)), np.int16)
        idx16[e_ipart[m], e_icol[m]] = e_srclocal[m]
        rel_all = np.full((P, max(c_tot, 1)), -1.0, np.float16)
        rel_all[e_r[m], e_acol[m]] = e_rel[m]
        per_core.append((np.ascontiguousarray(np.tile(idx16, (8, 1))),
                         np.ascontiguousarray(rel_all)))
    return meta, per_core


def _preprocess(x, edge_index, pos_encoding, W_gcn, b_gcn, W_pos, b_pos,
                n_nodes, n_cores):
    src = np.asarray(edge_index[0], dtype=np.int64)
    dst = np.asarray(edge_index[1], dtype=np.int64)
    meta, per_core_idx = _plan(src, dst, n_nodes, n_cores)
    npc, n_tiles = meta["npc"], meta["n_tiles"]
    nodes_pad = n_tiles * P
    dinv = meta["dinv"]

    x_s = np.zeros((meta["xs_rows"], ROW), np.float16)
    x_s[:n_nodes, :D] = (np.asarray(x, np.float32)
                         * dinv[:, None]).astype(np.float16)

    pos_f = np.asarray(pos_encoding, np.float32)
    node_tile, node_row = meta["node_tile"], meta["node_row"]
    # kernel output row (core c, tile t, row r) -> node id
    row_of_node = node_tile * P + node_row  # padded row within core
    per_core = []
    for c in range(n_cores):
        idx_arr, rel_all = per_core_idx[c]
        nodes = np.arange(c * npc, (c + 1) * npc)
        rpad = row_of_node[nodes]
        dv = np.zeros(nodes_pad, np.float32)
        dv[rpad] = dinv[nodes]
        dinv_tiles = np.ascontiguousarray(dv.reshape(n_tiles, P).T)
        pa = np.zeros((65, nodes_pad), np.float16)
        pa[:D, rpad] = pos_f[nodes].T.astype(np.float16)
        pa[D, rpad] = 1.0
        xo_full = np.zeros((nodes_pad, D), np.float16)
        xo_full[rpad] = x_s[nodes][:, :D]
        xo = np.ascontiguousarray(
            xo_full.reshape(n_tiles, P, D).transpose(1, 0, 2).reshape(
                P, n_tiles * D))
        per_core.append(dict(idx_all=idx_arr, rel_all=rel_all,
                             dinv_tiles=dinv_tiles,
                             posT=np.ascontiguousarray(pa),
                             xown=xo))

    b_sum = np.asarray(b_gcn, np.float32) + np.asarray(b_pos, np.float32)
    W_aug = np.zeros((65, D), np.float16)
    W_aug[:D] = np.asarray(W_pos, np.float32).astype(np.float16)
    W_aug[D] = b_sum.astype(np.float16)
    Wg16 = np.asarray(W_gcn, np.float32).astype(np.float16)

    shared = dict(x_s=x_s, W_gcn=Wg16, W_aug=W_aug)
    meta["row_of_node"] = row_of_node
    return shared, per_core, meta


def _build_program(meta):
    npc, n_tiles = meta["npc"], meta["n_tiles"]
    nodes_pad = n_tiles * P
    c_tot = meta["c_tot"]
    stat, B, NI, nch_call = (meta["stat"], meta["B"], meta["NI"],
                             meta["nch_call"])
    colbase, total_cols = meta["colbase"], meta["total_cols"]
    k0, novl = meta["k0"], meta["novl"]
    tile_ncol, tile_coloff = meta["tile_ncol"], meta["tile_coloff"]
    seg_rows, n_groups = meta["seg_rows"], meta["n_groups"]
    max_ncol = int(tile_ncol.max())
    nch_max = int(nch_call.max())

    nc = bacc.Bacc("TRN2", target_bir_lowering=False, debug=False,
                   num_swdge_queues=4)
    xs_d = nc.declare_dram_parameter(
        "x_s", [meta["xs_rows"], ROW], F16, isOutput=False)
    idx_d = nc.declare_dram_parameter(
        "idx_all", [P, max(total_cols, 1)], I16, isOutput=False)
    rel_d = nc.declare_dram_parameter(
        "rel_all", [P, max(c_tot, 1)], F16, isOutput=False)
    dinv_d = nc.declare_dram_parameter(
        "dinv_tiles", [P, n_tiles], F32, isOutput=False)
    posT_d = nc.declare_dram_parameter(
        "posT", [65, nodes_pad], F16, isOutput=False)
    xo_d = nc.declare_dram_parameter(
        "xown", [P, n_tiles * D], F16, isOutput=False)
    wg_d = nc.declare_dram_parameter("W_gcn", [D, D], F16, isOutput=False)
    wa_d = nc.declare_dram_parameter("W_aug", [65, D], F16, isOutput=False)
    out_d = nc.declare_dram_parameter("out", [nodes_pad, D], F32,
                                      isOutput=True)

    with tile.TileContext(nc) as tc:
        with (
            tc.tile_pool(name="const", bufs=1) as cpool,
            tc.tile_pool(name="msg", bufs=3) as mpool,
            tc.tile_pool(name="amat", bufs=3) as apool,
            tc.tile_pool(name="small", bufs=3) as spool,
            tc.tile_pool(name="outb", bufs=3) as opool,
            tc.tile_pool(name="ps_s", bufs=2, space="PSUM") as ps_s,
            tc.tile_pool(name="ps_t", bufs=2, space="PSUM") as ps_t,
            tc.tile_pool(name="ps_o", bufs=2, space="PSUM") as ps_o,
        ):
            iota_i = cpool.tile([P, P], mybir.dt.int16)
            nc.gpsimd.iota(iota_i[:], pattern=[[1, P]], base=0,
                           channel_multiplier=0)
            nc.gpsimd.load_library(mlp)
            iota_t = cpool.tile([P, P], F16)
            nc.vector.tensor_copy(out=iota_t[:], in_=iota_i[:])
            ident_t = cpool.tile([P, P], F16)
            make_identity(nc, ident_t[:])
            idx_ts = []
            for g in range(n_groups):
                g_lo = int(colbase[g, 0])
                g_cols = int(_cdiv(NI[g], 16).sum())
                it = cpool.tile([P, max(g_cols, 1)], I16, name=f"idxg{g}")
                nc.sync.dma_start(out=it[:],
                                  in_=idx_d[:, g_lo:g_lo + g_cols])
                idx_ts.append((it, g_lo))
            wg_t = cpool.tile([D, D], F16)
            nc.sync.dma_start(out=wg_t[:], in_=wg_d[:])
            wa_t = cpool.tile([65, D], F16)
            nc.sync.dma_start(out=wa_t[:], in_=wa_d[:])
            dinv_t = cpool.tile([P, n_tiles], F32)
            nc.sync.dma_start(out=dinv_t[:], in_=dinv_d[:])
            posT_t = cpool.tile([65, nodes_pad], F16)
            nc.sync.dma_start(out=posT_t[:], in_=posT_d[:])
            xo_t = cpool.tile([P, n_tiles * D], F16)
            nc.sync.dma_start(out=xo_t[:], in_=xo_d[:])
            rel_all = cpool.tile([P, max(c_tot, 1)], F16)
            nc.sync.dma_start(out=rel_all[:], in_=rel_d[:])

            for g in range(n_groups):
                t_lo = g * GROUP
                t_hi = min((g + 1) * GROUP, n_tiles)
                msg_of_s = {}
                for s in range(NSEG):
                    ni = int(NI[g, s])
                    nch = int(nch_call[g, s])
                    msg = mpool.tile([P, nch_max, ROW], F16, tag=f"m{s}",
                                     name=f"msg{s}")
                    if ni < nch * P:
                        nc.scalar.memzero(msg[:, nch - 1, :])
                    it, g_lo = idx_ts[g]
                    cb = int(colbase[g, s]) - g_lo
                    nc.gpsimd.dma_gather(
                        msg[:, :nch, :],
                        xs_d[s * seg_rows:(s + 1) * seg_rows, :],
                        it[:, cb:cb + _cdiv(ni, 16)],
                        ni,
                        ni,
                        ROW,
                        elem_step=ROW,
                        single_packet=False,
                        queue_num=s,
                    )
                    msg_of_s[s] = msg

                for t in range(t_lo, t_hi):
                    ncol = int(tile_ncol[t])
                    j0 = int(tile_coloff[t])
                    a_big = apool.tile([P, max_ncol, P], F16, tag="a_big")
                    nc.vector.tensor_tensor(
                        out=a_big[:, :ncol, :],
                        in0=rel_all[:, j0:j0 + ncol].unsqueeze(2)
                            .to_broadcast([P, ncol, P]),
                        in1=iota_t[:].unsqueeze(1).to_broadcast([P, ncol, P]),
                        op=eq)
                    psum_s = ps_s.tile([P, D], F32)
                    j = 0
                    for s in range(NSEG):
                        kk0 = int(k0[t, s])
                        for k in range(int(novl[t, s])):
                            nc.tensor.matmul(
                                out=psum_s[:],
                                lhsT=a_big[:, j, :],
                                rhs=msg_of_s[s][:, kk0 + k, :D],
                                start=(j == 0), stop=(j == ncol - 1))
                            j += 1

                    ssum = spool.tile([P, D], F32, tag="ssum")
                    nc.vector.tensor_tensor(
                        out=ssum[:], in0=psum_s[:],
                        in1=xo_t[:, t * D:(t + 1) * D], op=add_op)
                    s16 = spool.tile([P, D], F16, tag="s16")
                    nc.scalar.mul(out=s16[:], in_=ssum[:],
                                  mul=dinv_t[:, t:t + 1])
                    psT = ps_t.tile([D, P], F16)
                    nc.tensor.transpose(out=psT[:], in_=s16[:],
                                        identity=ident_t[:])
                    sT = spool.tile([D, P], F16, tag="sT")
                    nc.scalar.copy(out=sT[:], in_=psT[:])
                    psum_o = ps_o.tile([P, D], F32)
                    nc.tensor.matmul(out=psum_o[:], lhsT=sT[:], rhs=wg_t[:],
                                     start=True, stop=False)
                    nc.tensor.matmul(out=psum_o[:],
                                     lhsT=posT_t[:, t * P:(t + 1) * P],
                                     rhs=wa_t[:], start=False, stop=True)
                    out_sb = opool.tile([P, D], F32)
                    nc.scalar.copy(out=out_sb[:], in_=psum_o[:])
                    nc.sync.dma_start(out=out_d[t * P:(t + 1) * P, :],
                                      in_=out_sb[:])
    nc.compile()
    return nc


def kernel(x, edge_index, pos_encoding, W_gcn, b_gcn, W_pos, b_pos,
           _trace=False, _result_box=None, _tmpdir=None,
           _n_nodes=N_NODES, _n_cores=N_CORES):
    shared, per_core, meta = _preprocess(
        x, edge_index, pos_encoding, W_gcn, b_gcn, W_pos, b_pos,
        _n_nodes, _n_cores)
    nc = _build_program(meta)
    in_maps = [{**shared, **per_core[c]} for c in range(_n_cores)]
    res = run_bass_kernel_spmd(nc, in_maps, list(range(_n_cores)),
                               trace=_trace, tmpdir=_tmpdir)
    if _result_box is not None:
        _result_box.append(res)
    npc = meta["npc"]
    row_of_node = meta["row_of_node"]
    out = np.empty((_n_nodes, D), np.float32)
    for c in range(_n_cores):
        nodes = np.arange(c * npc, (c + 1) * npc)
        out[nodes] = np.asarray(res.results[c]["out"],
                                np.float32)[row_of_node[nodes]]
    return out


if __name__ == "__main__":
    rng = np.random.default_rng(0)
    x = rng.standard_normal((N_NODES, D), dtype=np.float32)
    ei = rng.integers(0, N_NODES, size=(2, 1600000)).astype(np.int64)
    pe = rng.standard_normal((N_NODES, D), dtype=np.float32)
    Wg = rng.standard_normal((D, D), dtype=np.float32) / 8
    bg = rng.standard_normal(D, dtype=np.float32) * 0.01
    Wp = rng.standard_normal((D, D), dtype=np.float32) / 8
    bp = rng.standard_normal(D, dtype=np.float32) * 0.01
    out = kernel(x, ei, pe, Wg, bg, Wp, bp)
    print(out.shape, out.dtype)

